# revision 16
# baseline (speedup 1.0000x reference)
"""AdaptivePatchEmbedding Trainium2 kernel (8 NeuronCores, data-parallel).

Reference computation (see problem):
  xr = x.reshape(N, R, 48); logits = relu(xr@W1+b1)@W2+b2
  g = -log(-log(u+1e-10)+1e-10); argmax over (logits+g) -> hard one-hot
  out[n,r,t,:] = patches_{e}[n,r,t,:] @ W_emb_e  (e = selected expert)
  x_patch = out.reshape(N, 42, 512) + pos_emb;  cls_pred = argmax.T.flatten()

Device strategy per core (512 rows of N=4096):
  - Router in fp32 (argmax must match reference bit-for-bit): hT = W1.T @ xrT
    (K-major x, host-transposed), ACT relu+b1, y0 = h@W2 per 128-row chunk,
    + host-computed gumbel (b2 folded), argmax via DVE max/max_index.
  - Selection: the winning expert index per row is transposed (TensorE,
    [128,1]->[1,128]) and partition-broadcast (GpSimd) to [128,n]; the mask
    multiply is fused on DVE: X~ = (idx == expert_of_partition) * xrT.
  - Embedding: one-hot folded into data: X~[193, n] = stacked per-expert masked
    copies of xrT (+ ones row).  out = X~.T @ G_t with G host-precomputed
    ([expert-placed W_emb rows] + pos_emb row), in float32r (1 cyc/row, ~1.5e-4).
    X~ rounded to f32r on ACT.  Two accumulating matmuls (K=128+65) per
    (tile, t); PSUM: 5 banks output rotation + router banks.
"""

import numpy as np
import ml_dtypes

# ---- problem constants (hardcoded; kernel.py must be self-contained) ----
B, C, S = 128, 32, 336
N_TOT = B * C            # 4096
R = S // 48              # 7
T = 6                    # patches per region after repeat
D = 512
N_CORES = 8
N_PER = N_TOT // N_CORES  # 512
NCHUNK = N_PER // 128     # 4
NJ = R * NCHUNK           # 28 tiles of 128 rows (row = r*512 + n_local)
PATCH_LENS = [8, 16, 24, 48]

_CACHE = {}


def _pos_emb(L, d):
    pos = np.arange(L, dtype=np.float64)[:, None]
    div = np.exp(np.arange(0, d, 2, dtype=np.float64) * -(np.log(10000.0) / d))
    pe = np.zeros((L, d), np.float32)
    pe[:, 0::2] = np.sin(pos * div).astype(np.float32)
    pe[:, 1::2] = np.cos(pos * div).astype(np.float32)
    return pe


def _repeat_q():
    q = np.zeros((4, T), np.int64)
    for e, plen in enumerate(PATCH_LENS):
        n = 48 // plen
        k = T - n + 1
        idx = np.arange(T) // k if k > 1 else np.arange(T)
        q[e] = np.minimum(idx, n - 1)
    return q


def _build():
    import concourse.bacc as bacc
    import concourse.mybir as mybir
    import concourse.tile as tile
    from concourse.masks import make_identity

    f32 = mybir.dt.float32
    f32r = mybir.dt.float32r
    bf16 = mybir.dt.bfloat16
    i32 = mybir.dt.int32
    u32 = mybir.dt.uint32

    nc = bacc.Bacc(trn_type="TRN2", target_bir_lowering=False)

    xa_d = nc.dram_tensor("xa", [128, NJ * 128], f32, kind="ExternalInput")
    xb_d = nc.dram_tensor("xb", [64, NJ * 128], bf16, kind="ExternalInput")
    g_d = nc.dram_tensor("g", [128, NJ, 4], f32, kind="ExternalInput")
    w1_d = nc.dram_tensor("w1", [48, 64], f32, kind="ExternalInput")
    b1_d = nc.dram_tensor("b1", [64, 1], f32, kind="ExternalInput")
    w2_d = nc.dram_tensor("w2", [64, 4], f32, kind="ExternalInput")
    ga_d = nc.dram_tensor("ga", [128, T, D], bf16, kind="ExternalInput")
    gb_d = nc.dram_tensor("gb", [66, R * T, D], bf16, kind="ExternalInput")
    ea_d = nc.dram_tensor("ea", [128, 1], f32, kind="ExternalInput")
    id_d = nc.dram_tensor("idm", [128, 128], bf16, kind="ExternalInput")
    eb_d = nc.dram_tensor("eb", [64, 1], f32, kind="ExternalInput")

    xp_d = nc.dram_tensor("xp", [N_PER, R * T, D], f32, kind="ExternalOutput")
    cls_d = nc.dram_tensor("cls", [R, N_PER], i32, kind="ExternalOutput")

    with tile.TileContext(nc) as tc:
        with (
            tc.tile_pool(name="sb", bufs=1) as sb,
            tc.tile_pool(name="sw", bufs=2) as sw,
            tc.tile_pool(name="ps", bufs=1, space="PSUM") as ps,
        ):
            # ---- resident loads (input ring = ACT HWDGE; outputs on SP ring) ----
            xa_s = sb.tile([128, NJ * 128], f32)
            w1_s = sb.tile([48, 64], f32)
            b1_s = sb.tile([64, 1], f32)
            w2_s = sb.tile([64, 4], f32)
            g_s = sb.tile([128, NJ, 4], f32)
            ea_s = sb.tile([128, 1], f32)
            eb_s = sb.tile([64, 1], f32)
            xb_s = sb.tile([64, NJ * 128], bf16)
            ga_s = sb.tile([128, T, D], bf16)
            ident = sb.tile([128, 128], bf16, name="ident")
            gb_all = sb.tile([66, R * T, D], bf16)
            qtr = NJ * 128 // 4
            nc.gpsimd.dma_start(xa_s[:, 0:qtr], xa_d[:, 0:qtr])
            nc.gpsimd.dma_start(w1_s[:], w1_d[:])
            nc.gpsimd.dma_start(b1_s[:], b1_d[:])
            nc.gpsimd.dma_start(w2_s[:], w2_d[:])
            nc.gpsimd.dma_start(ident[:], id_d[:])
            nc.gpsimd.dma_start(g_s[:], g_d[:])
            nc.gpsimd.dma_start(ea_s[:], ea_d[:])
            nc.gpsimd.dma_start(eb_s[:], eb_d[:])
            for _q in range(1, 4):
                nc.gpsimd.dma_start(
                    xa_s[:, _q * qtr:(_q + 1) * qtr], xa_d[:, _q * qtr:(_q + 1) * qtr]
                )
            nc.gpsimd.dma_start(ga_s[:], ga_d[:])
            nc.gpsimd.dma_start(xb_s[:], xb_d[:])
            nc.gpsimd.dma_start(gb_all[:], gb_d[:])

            idx_all = sb.tile([1, NJ * 128], bf16)
            cls_sb = sb.tile([128, NJ], i32)
            hT = sb.tile([64, NJ * 128], f32)

            # ---- router: fused m1 + per-j argmax chain per 512-chunk ----
            # ---- embed tiles ----
            def embed_tile(j):
                    r, cn = j // NCHUNK, j % NCHUNK
                    sl = slice(j * 128, (j + 1) * 128)
                    idxb = sw.tile([128, 128], bf16, tag="idxb", bufs=4, name=f"ib{j}")
                    nc.gpsimd.partition_broadcast(idxb[:], idx_all[0:1, sl])
                    xta = sw.tile([128, 128], bf16, tag="xta", bufs=4, name=f"xa{j}")
                    nc.vector.scalar_tensor_tensor(
                        xta[:], idxb[:], ea_s[:, 0:1], xa_s[:, sl],
                        mybir.AluOpType.is_equal, mybir.AluOpType.mult,
                    )
                    xtb = sw.tile([66, 128], bf16, tag="xtb", bufs=4, name=f"xb{j}")
                    nc.vector.scalar_tensor_tensor(
                        xtb[0:64, :], idxb[0:64, :], eb_s[:, 0:1],
                        xb_s[:, sl],
                        mybir.AluOpType.is_equal, mybir.AluOpType.mult,
                    )
                    nc.vector.memset(xtb[64:66, :], 1.0)
                    out_sb = sw.tile([128, T, D], f32, tag="osb", bufs=6, name=f"os{j}")
                    for t in range(T):
                        o_t = ps.tile([128, D], f32, tag="pout", bufs=4,
                                      name=f"ops{j}_{t}")
                        nc.tensor.matmul(
                            o_t[:], xta[:], ga_s[:, t, :],
                            start=True, stop=False,
                        )
                        nc.tensor.matmul(
                            o_t[:], xtb[:], gb_all[:, r * T + t, :],
                            start=False, stop=True,
                        )
                        if t in (0, 3):
                            nc.vector.tensor_copy(out_sb[:, t, :], o_t[:])
                        else:
                            nc.scalar.copy(out_sb[:, t, :], o_t[:])
                    out_eng = nc.sync if (j % 2 == 0) else nc.scalar
                    out_eng.dma_start(
                        xp_d[cn * 128:(cn + 1) * 128, r * T:(r + 1) * T, :],
                        out_sb[:],
                    )


            routers = []
            with nc.named_scope("router"):
              for jc in range(7):
                h_ps = ps.tile([64, 512], f32, tag="pout", bufs=4, name=f"hps{jc}")
                nc.tensor.matmul(
                    h_ps[:], w1_s[:], xa_s[0:48, jc * 512:(jc + 1) * 512],
                    start=True, stop=True,
                )
                nc.scalar.activation(
                    hT[:, jc * 512:(jc + 1) * 512], h_ps[:],
                    mybir.ActivationFunctionType.Relu, bias=b1_s[:, 0:1],
                )
                for j in range(jc * 4, jc * 4 + 4):
                    sl = slice(j * 128, (j + 1) * 128)
                    y0 = ps.tile([128, 4], f32, tag="pa", bufs=3, name=f"y0_{j}")
                    nc.tensor.matmul(
                        y0[:], hT[:, sl], w2_s[:], start=True, stop=True
                    )
                    y8 = sw.tile([128, 8], f32, tag="y8", bufs=4, name=f"y8_{j}")
                    nc.vector.memset(y8[:, 4:8], -1e30)
                    nc.vector.tensor_add(y8[:, 0:4], y0[:], g_s[:, j, :])
                    vmax = sw.tile([128, 8], f32, tag="vmax", bufs=4, name=f"vm{j}")
                    nc.vector.max(vmax[:], y8[:])
                    idx8 = sw.tile([128, 8], u32, tag="idx", bufs=4, name=f"ix{j}")
                    nc.vector.max_index(idx8[:], vmax[:], y8[:])
                    nc.vector.tensor_copy(
                        cls_sb[:, j:j + 1], idx8[:, 0:1].bitcast(i32)
                    )
                    idxf = sw.tile([128, 1], bf16, tag="idxf", bufs=4, name=f"if{j}")
                    nc.vector.tensor_copy(idxf[:], idx8[:, 0:1].bitcast(i32))
                    ir_ps = ps.tile([1, 128], bf16, tag="pb", bufs=1, name=f"ir{j}")
                    nc.tensor.transpose(ir_ps[:], idxf[:], ident[:])
                    nc.vector.tensor_copy(idx_all[:, sl], ir_ps[:])
                if jc >= 2:
                    for _j in range((jc - 2) * 4, (jc - 1) * 4):
                        embed_tile(_j)

            with nc.named_scope("embed_drive"):
                for _j in range(20, NJ):
                    embed_tile(_j)
            nc.gpsimd.dma_start(
                cls_d.rearrange("r (c p) -> p r c", p=128),
                cls_sb[:].rearrange("p (r c) -> p r c", r=R),
            )

    nc.compile()
    return nc


def _host_prep(x, u_noise, W1, b1, W2, b2, W_embs):
    """Build per-core and shared input arrays."""
    q = _repeat_q()
    pos = _pos_emb(R * T, D)

    G = np.zeros((T, 193, D), np.float32)
    for e, plen in enumerate(PATCH_LENS):
        for t in range(T):
            r0 = e * 48 + q[e, t] * plen
            G[t, r0:r0 + plen, :] = W_embs[e]
    GA = np.ascontiguousarray(
        G[:, :128, :].transpose(1, 0, 2)
    ).astype(ml_dtypes.bfloat16)                      # [128, T, D]
    pos_hi = pos.astype(ml_dtypes.bfloat16)
    pos_lo = (pos - pos_hi.astype(np.float32)).astype(ml_dtypes.bfloat16)
    GB = np.zeros((66, R * T, D), ml_dtypes.bfloat16)  # [66, R*T, D]
    for r in range(R):
        for t in range(T):
            GB[:64, r * T + t] = G[t, 128:192].astype(ml_dtypes.bfloat16)
            GB[64, r * T + t] = pos_hi[r * T + t]
            GB[65, r * T + t] = pos_lo[r * T + t]

    eA = np.concatenate([np.full(48, 0), np.full(48, 1), np.full(32, 2)])
    eAc = np.ascontiguousarray(eA.astype(np.float32).reshape(128, 1))
    eB = np.concatenate([np.full(16, 2), np.full(48, 3)])
    eBc = np.ascontiguousarray(eB.astype(np.float32).reshape(64, 1))

    g_full = -np.log(-np.log(u_noise + np.float32(1e-10)) + np.float32(1e-10))
    g_full = (g_full + b2[None, None, :]).astype(np.float32)  # [N, R, 4]

    xs = np.ascontiguousarray(x.reshape(N_TOT, R, 48))
    kA = np.concatenate([np.arange(48), np.arange(48), np.arange(32)])
    kB = np.concatenate([np.arange(32, 48), np.arange(48)])

    in_maps = []
    for c in range(N_CORES):
        xc = xs[c * N_PER:(c + 1) * N_PER]            # [512, 7, 48]
        xa = np.ascontiguousarray(
            xc[:, :, kA].transpose(2, 1, 0).reshape(128, NJ * 128)
        )
        xb = np.ascontiguousarray(
            xc[:, :, kB].transpose(2, 1, 0).reshape(64, NJ * 128)
        ).astype(ml_dtypes.bfloat16)
        gc = g_full[c * N_PER:(c + 1) * N_PER]        # [512, 7, 4]
        gcr = np.ascontiguousarray(
            gc.transpose(1, 0, 2).reshape(NJ, 128, 4).transpose(1, 0, 2)
        )                                             # [128, NJ, 4]
        in_maps.append({
            "xa": xa, "xb": xb, "g": gcr,
            "idm": np.eye(128, dtype=ml_dtypes.bfloat16),
            "w1": np.ascontiguousarray(W1),
            "b1": np.ascontiguousarray(b1.reshape(64, 1)),
            "w2": np.ascontiguousarray(W2),
            "ga": GA, "gb": GB, "ea": eAc, "eb": eBc,
        })
    return in_maps


def kernel(x, u_noise, W1, b1, W2, b2, W_emb0, W_emb1, W_emb2, W_emb3,
           _want_trace=False):
    from concourse.bass_utils import run_bass_kernel_spmd

    x = np.asarray(x, np.float32)
    u_noise = np.asarray(u_noise, np.float32)
    W1 = np.asarray(W1, np.float32)
    b1 = np.asarray(b1, np.float32)
    W2 = np.asarray(W2, np.float32)
    b2 = np.asarray(b2, np.float32)
    W_embs = [np.asarray(w, np.float32) for w in (W_emb0, W_emb1, W_emb2, W_emb3)]

    if "nc" not in _CACHE:
        _CACHE["nc"] = _build()
    nc = _CACHE["nc"]

    in_maps = _host_prep(x, u_noise, W1, b1, W2, b2, W_embs)
    res = run_bass_kernel_spmd(
        nc, in_maps, core_ids=list(range(N_CORES)), trace=_want_trace
    )
    if _want_trace:
        _CACHE["last_result"] = res

    x_patch = np.concatenate(
        [res.results[c]["xp"] for c in range(N_CORES)], axis=0
    )
    cls_full = np.concatenate(
        [res.results[c]["cls"] for c in range(N_CORES)], axis=1
    )
    cls_pred = np.ascontiguousarray(cls_full.reshape(-1))
    return x_patch, np.int32(C), cls_pred


# revision 19
# speedup vs baseline: 1.0442x; 1.0442x over previous
"""AdaptivePatchEmbedding Trainium2 kernel (8 NeuronCores, data-parallel).

Reference computation (see problem):
  xr = x.reshape(N, R, 48); logits = relu(xr@W1+b1)@W2+b2
  g = -log(-log(u+1e-10)+1e-10); argmax over (logits+g) -> hard one-hot
  out[n,r,t,:] = patches_{e}[n,r,t,:] @ W_emb_e  (e = selected expert)
  x_patch = out.reshape(N, 42, 512) + pos_emb;  cls_pred = argmax.T.flatten()

Device strategy per core (512 rows of N=4096):
  - Router in fp32 (argmax must match reference bit-for-bit): hT = W1.T @ xrT
    (K-major x, host-transposed), ACT relu+b1, y0 = h@W2 per 128-row chunk,
    + host-computed gumbel (b2 folded), argmax via DVE max/max_index.
  - Selection: the winning expert index per row is transposed (TensorE,
    [128,1]->[1,128]) and partition-broadcast (GpSimd) to [128,n]; the mask
    multiply is fused on DVE: X~ = (idx == expert_of_partition) * xrT.
  - Embedding: one-hot folded into data: X~[193, n] = stacked per-expert masked
    copies of xrT (+ ones row).  out = X~.T @ G_t with G host-precomputed
    ([expert-placed W_emb rows] + pos_emb row), in float32r (1 cyc/row, ~1.5e-4).
    X~ rounded to f32r on ACT.  Two accumulating matmuls (K=128+65) per
    (tile, t); PSUM: 5 banks output rotation + router banks.
"""

import numpy as np
import ml_dtypes

# ---- problem constants (hardcoded; kernel.py must be self-contained) ----
B, C, S = 128, 32, 336
N_TOT = B * C            # 4096
R = S // 48              # 7
T = 6                    # patches per region after repeat
D = 512
N_CORES = 8
N_PER = N_TOT // N_CORES  # 512
NCHUNK = N_PER // 128     # 4
NJ = R * NCHUNK           # 28 tiles of 128 rows (row = r*512 + n_local)
PATCH_LENS = [8, 16, 24, 48]

_CACHE = {}


def _pos_emb(L, d):
    pos = np.arange(L, dtype=np.float64)[:, None]
    div = np.exp(np.arange(0, d, 2, dtype=np.float64) * -(np.log(10000.0) / d))
    pe = np.zeros((L, d), np.float32)
    pe[:, 0::2] = np.sin(pos * div).astype(np.float32)
    pe[:, 1::2] = np.cos(pos * div).astype(np.float32)
    return pe


def _repeat_q():
    q = np.zeros((4, T), np.int64)
    for e, plen in enumerate(PATCH_LENS):
        n = 48 // plen
        k = T - n + 1
        idx = np.arange(T) // k if k > 1 else np.arange(T)
        q[e] = np.minimum(idx, n - 1)
    return q


def _build():
    import concourse.bacc as bacc
    import concourse.mybir as mybir
    import concourse.tile as tile
    from concourse.masks import make_identity

    f32 = mybir.dt.float32
    f32r = mybir.dt.float32r
    bf16 = mybir.dt.bfloat16
    i32 = mybir.dt.int32
    u32 = mybir.dt.uint32

    nc = bacc.Bacc(trn_type="TRN2", target_bir_lowering=False)

    xa_d = nc.dram_tensor("xa", [128, NJ * 128], bf16, kind="ExternalInput")
    xal_d = nc.dram_tensor("xal", [48, NJ * 128], bf16, kind="ExternalInput")
    xb_d = nc.dram_tensor("xb", [64, NJ * 128], bf16, kind="ExternalInput")
    g_d = nc.dram_tensor("g", [128, NJ, 4], f32, kind="ExternalInput")
    w1h_d = nc.dram_tensor("w1h", [48, 64], bf16, kind="ExternalInput")
    w1l_d = nc.dram_tensor("w1l", [48, 64], bf16, kind="ExternalInput")
    b1_d = nc.dram_tensor("b1", [64, 1], f32, kind="ExternalInput")
    w2_d = nc.dram_tensor("w2", [64, 4], f32, kind="ExternalInput")
    ga_d = nc.dram_tensor("ga", [128, T, D], bf16, kind="ExternalInput")
    gb_d = nc.dram_tensor("gb", [66, R * T, D], bf16, kind="ExternalInput")
    ea_d = nc.dram_tensor("ea", [128, 1], f32, kind="ExternalInput")
    id_d = nc.dram_tensor("idm", [128, 128], bf16, kind="ExternalInput")
    eb_d = nc.dram_tensor("eb", [64, 1], f32, kind="ExternalInput")

    xp_d = nc.dram_tensor("xp", [N_PER, R * T, D], f32, kind="ExternalOutput")
    cls_d = nc.dram_tensor("cls", [R, N_PER], i32, kind="ExternalOutput")

    with tile.TileContext(nc) as tc:
        with (
            tc.tile_pool(name="sb", bufs=1) as sb,
            tc.tile_pool(name="sw", bufs=2) as sw,
            tc.tile_pool(name="ps", bufs=1, space="PSUM") as ps,
        ):
            # ---- resident loads (input ring = ACT HWDGE; outputs on SP ring) ----
            xa_s = sb.tile([128, NJ * 128], bf16)
            xal_s = sb.tile([48, NJ * 128], bf16)
            w1h_s = sb.tile([48, 64], bf16)
            w1l_s = sb.tile([48, 64], bf16)
            b1_s = sb.tile([64, 1], f32)
            w2_s = sb.tile([64, 4], f32)
            g_s = sb.tile([128, NJ, 4], f32)
            ea_s = sb.tile([128, 1], f32)
            eb_s = sb.tile([64, 1], f32)
            xb_s = sb.tile([64, NJ * 128], bf16)
            ga_s = sb.tile([128, T, D], bf16)
            ident = sb.tile([128, 128], bf16, name="ident")
            gb_all = sb.tile([66, R * T, D], bf16)
            qtr = NJ * 128 // 4
            nc.gpsimd.dma_start(xa_s[:, 0:qtr], xa_d[:, 0:qtr])
            nc.gpsimd.dma_start(w1h_s[:], w1h_d[:])
            nc.gpsimd.dma_start(w1l_s[:], w1l_d[:])
            nc.gpsimd.dma_start(b1_s[:], b1_d[:])
            nc.gpsimd.dma_start(w2_s[:], w2_d[:])
            nc.gpsimd.dma_start(ident[:], id_d[:])
            nc.gpsimd.dma_start(g_s[:], g_d[:])
            nc.gpsimd.dma_start(ea_s[:], ea_d[:])
            nc.gpsimd.dma_start(eb_s[:], eb_d[:])
            for _q in range(1, 4):
                nc.gpsimd.dma_start(
                    xa_s[:, _q * qtr:(_q + 1) * qtr], xa_d[:, _q * qtr:(_q + 1) * qtr]
                )
            nc.gpsimd.dma_start(xal_s[:], xal_d[:])
            nc.gpsimd.dma_start(ga_s[:], ga_d[:])
            nc.gpsimd.dma_start(xb_s[:], xb_d[:])
            nc.gpsimd.dma_start(gb_all[:], gb_d[:])

            idx_all = sb.tile([1, NJ * 128], bf16)
            cls_sb = sb.tile([128, NJ], i32)
            hT = sb.tile([64, NJ * 128], f32)

            # ---- router: fused m1 + per-j argmax chain per 512-chunk ----
            with nc.named_scope("router"):
              for jc in range(7):
                h_ps = ps.tile([64, 512], f32, tag="pout", bufs=4, name=f"hps{jc}")
                ch = slice(jc * 512, (jc + 1) * 512)
                nc.tensor.matmul(
                    h_ps[:], w1h_s[:], xa_s[0:48, ch], start=True, stop=False
                )
                nc.tensor.matmul(
                    h_ps[:], w1l_s[:], xa_s[0:48, ch], start=False, stop=False
                )
                nc.tensor.matmul(
                    h_ps[:], w1h_s[:], xal_s[:, ch], start=False, stop=True
                )
                nc.scalar.activation(
                    hT[:, jc * 512:(jc + 1) * 512], h_ps[:],
                    mybir.ActivationFunctionType.Relu, bias=b1_s[:, 0:1],
                )
                for j in range(jc * 4, jc * 4 + 4):
                    sl = slice(j * 128, (j + 1) * 128)
                    y0 = ps.tile([128, 4], f32, tag="pa", bufs=3, name=f"y0_{j}")
                    nc.tensor.matmul(
                        y0[:], hT[:, sl], w2_s[:], start=True, stop=True
                    )
                    y8 = sw.tile([128, 8], f32, tag="y8", bufs=4, name=f"y8_{j}")
                    nc.vector.memset(y8[:, 4:8], -1e30)
                    nc.vector.tensor_add(y8[:, 0:4], y0[:], g_s[:, j, :])
                    vmax = sw.tile([128, 8], f32, tag="vmax", bufs=4, name=f"vm{j}")
                    nc.vector.max(vmax[:], y8[:])
                    idx8 = sw.tile([128, 8], u32, tag="idx", bufs=4, name=f"ix{j}")
                    nc.vector.max_index(idx8[:], vmax[:], y8[:])
                    nc.vector.tensor_copy(
                        cls_sb[:, j:j + 1], idx8[:, 0:1].bitcast(i32)
                    )
                    idxf = sw.tile([128, 1], bf16, tag="idxf", bufs=4, name=f"if{j}")
                    nc.vector.tensor_copy(idxf[:], idx8[:, 0:1].bitcast(i32))
                    ir_ps = ps.tile([1, 128], bf16, tag="pb", bufs=1, name=f"ir{j}")
                    nc.tensor.transpose(ir_ps[:], idxf[:], ident[:])
                    nc.vector.tensor_copy(idx_all[:, sl], ir_ps[:])

            # ---- embed tiles ----
            with nc.named_scope("embed"):
                for j in range(NJ):
                    r, cn = j // NCHUNK, j % NCHUNK
                    sl = slice(j * 128, (j + 1) * 128)
                    idxb = sw.tile([128, 128], bf16, tag="idxb", bufs=4, name=f"ib{j}")
                    nc.gpsimd.partition_broadcast(idxb[:], idx_all[0:1, sl])
                    xta = sw.tile([128, 128], bf16, tag="xta", bufs=4, name=f"xa{j}")
                    nc.vector.scalar_tensor_tensor(
                        xta[:], idxb[:], ea_s[:, 0:1], xa_s[:, sl],
                        mybir.AluOpType.is_equal, mybir.AluOpType.mult,
                    )
                    xtb = sw.tile([66, 128], bf16, tag="xtb", bufs=4, name=f"xb{j}")
                    nc.vector.scalar_tensor_tensor(
                        xtb[0:64, :], idxb[0:64, :], eb_s[:, 0:1],
                        xb_s[:, sl],
                        mybir.AluOpType.is_equal, mybir.AluOpType.mult,
                    )
                    nc.vector.memset(xtb[64:66, :], 1.0)
                    out_sb = sw.tile([128, T, D], f32, tag="osb", bufs=6, name=f"os{j}")
                    for t in range(T):
                        o_t = ps.tile([128, D], f32, tag="pout", bufs=4,
                                      name=f"ops{j}_{t}")
                        nc.tensor.matmul(
                            o_t[:], xta[:], ga_s[:, t, :],
                            start=True, stop=False,
                        )
                        nc.tensor.matmul(
                            o_t[:], xtb[:], gb_all[:, r * T + t, :],
                            start=False, stop=True,
                        )
                        if t in (0, 3):
                            nc.vector.tensor_copy(out_sb[:, t, :], o_t[:])
                        else:
                            nc.scalar.copy(out_sb[:, t, :], o_t[:])
                    out_eng = nc.sync if (j % 2 == 0) else nc.scalar
                    out_eng.dma_start(
                        xp_d[cn * 128:(cn + 1) * 128, r * T:(r + 1) * T, :],
                        out_sb[:],
                    )
            nc.gpsimd.dma_start(
                cls_d.rearrange("r (c p) -> p r c", p=128),
                cls_sb[:].rearrange("p (r c) -> p r c", r=R),
            )

    nc.compile()
    return nc


def _host_prep(x, u_noise, W1, b1, W2, b2, W_embs):
    """Build per-core and shared input arrays."""
    q = _repeat_q()
    pos = _pos_emb(R * T, D)

    G = np.zeros((T, 193, D), np.float32)
    for e, plen in enumerate(PATCH_LENS):
        for t in range(T):
            r0 = e * 48 + q[e, t] * plen
            G[t, r0:r0 + plen, :] = W_embs[e]
    GA = np.ascontiguousarray(
        G[:, :128, :].transpose(1, 0, 2)
    ).astype(ml_dtypes.bfloat16)                      # [128, T, D]
    pos_hi = pos.astype(ml_dtypes.bfloat16)
    pos_lo = (pos - pos_hi.astype(np.float32)).astype(ml_dtypes.bfloat16)
    GB = np.zeros((66, R * T, D), ml_dtypes.bfloat16)  # [66, R*T, D]
    for r in range(R):
        for t in range(T):
            GB[:64, r * T + t] = G[t, 128:192].astype(ml_dtypes.bfloat16)
            GB[64, r * T + t] = pos_hi[r * T + t]
            GB[65, r * T + t] = pos_lo[r * T + t]

    eA = np.concatenate([np.full(48, 0), np.full(48, 1), np.full(32, 2)])
    eAc = np.ascontiguousarray(eA.astype(np.float32).reshape(128, 1))
    eB = np.concatenate([np.full(16, 2), np.full(48, 3)])
    eBc = np.ascontiguousarray(eB.astype(np.float32).reshape(64, 1))

    g_full = -np.log(-np.log(u_noise + np.float32(1e-10)) + np.float32(1e-10))
    g_full = (g_full + b2[None, None, :]).astype(np.float32)  # [N, R, 4]

    xs = np.ascontiguousarray(x.reshape(N_TOT, R, 48))
    kA = np.concatenate([np.arange(48), np.arange(48), np.arange(32)])
    kB = np.concatenate([np.arange(32, 48), np.arange(48)])

    in_maps = []
    for c in range(N_CORES):
        xc = xs[c * N_PER:(c + 1) * N_PER]            # [512, 7, 48]
        xa32 = np.ascontiguousarray(
            xc[:, :, kA].transpose(2, 1, 0).reshape(128, NJ * 128)
        )
        xa = xa32.astype(ml_dtypes.bfloat16)
        xal = (xa32[0:48] - xa[0:48].astype(np.float32)).astype(ml_dtypes.bfloat16)
        xb = np.ascontiguousarray(
            xc[:, :, kB].transpose(2, 1, 0).reshape(64, NJ * 128)
        ).astype(ml_dtypes.bfloat16)
        gc = g_full[c * N_PER:(c + 1) * N_PER]        # [512, 7, 4]
        gcr = np.ascontiguousarray(
            gc.transpose(1, 0, 2).reshape(NJ, 128, 4).transpose(1, 0, 2)
        )                                             # [128, NJ, 4]
        W1h = W1.astype(ml_dtypes.bfloat16)
        W1l = (W1 - W1h.astype(np.float32)).astype(ml_dtypes.bfloat16)
        in_maps.append({
            "xa": xa, "xal": xal, "xb": xb, "g": gcr,
            "idm": np.eye(128, dtype=ml_dtypes.bfloat16),
            "w1h": np.ascontiguousarray(W1h),
            "w1l": np.ascontiguousarray(W1l),
            "b1": np.ascontiguousarray(b1.reshape(64, 1)),
            "w2": np.ascontiguousarray(W2),
            "ga": GA, "gb": GB, "ea": eAc, "eb": eBc,
        })
    return in_maps


def kernel(x, u_noise, W1, b1, W2, b2, W_emb0, W_emb1, W_emb2, W_emb3,
           _want_trace=False):
    from concourse.bass_utils import run_bass_kernel_spmd

    x = np.asarray(x, np.float32)
    u_noise = np.asarray(u_noise, np.float32)
    W1 = np.asarray(W1, np.float32)
    b1 = np.asarray(b1, np.float32)
    W2 = np.asarray(W2, np.float32)
    b2 = np.asarray(b2, np.float32)
    W_embs = [np.asarray(w, np.float32) for w in (W_emb0, W_emb1, W_emb2, W_emb3)]

    if "nc" not in _CACHE:
        _CACHE["nc"] = _build()
    nc = _CACHE["nc"]

    in_maps = _host_prep(x, u_noise, W1, b1, W2, b2, W_embs)
    res = run_bass_kernel_spmd(
        nc, in_maps, core_ids=list(range(N_CORES)), trace=_want_trace
    )
    if _want_trace:
        _CACHE["last_result"] = res

    x_patch = np.concatenate(
        [res.results[c]["xp"] for c in range(N_CORES)], axis=0
    )
    cls_full = np.concatenate(
        [res.results[c]["cls"] for c in range(N_CORES)], axis=1
    )
    cls_pred = np.ascontiguousarray(cls_full.reshape(-1))
    return x_patch, np.int32(C), cls_pred


# revision 20
# speedup vs baseline: 1.1267x; 1.0790x over previous
"""AdaptivePatchEmbedding Trainium2 kernel (8 NeuronCores, data-parallel).

Reference computation (see problem):
  xr = x.reshape(N, R, 48); logits = relu(xr@W1+b1)@W2+b2
  g = -log(-log(u+1e-10)+1e-10); argmax over (logits+g) -> hard one-hot
  out[n,r,t,:] = patches_{e}[n,r,t,:] @ W_emb_e  (e = selected expert)
  x_patch = out.reshape(N, 42, 512) + pos_emb;  cls_pred = argmax.T.flatten()

Device strategy per core (512 rows of N=4096):
  - Router in fp32 (argmax must match reference bit-for-bit): hT = W1.T @ xrT
    (K-major x, host-transposed), ACT relu+b1, y0 = h@W2 per 128-row chunk,
    + host-computed gumbel (b2 folded), argmax via DVE max/max_index.
  - Selection: the winning expert index per row is transposed (TensorE,
    [128,1]->[1,128]) and partition-broadcast (GpSimd) to [128,n]; the mask
    multiply is fused on DVE: X~ = (idx == expert_of_partition) * xrT.
  - Embedding: one-hot folded into data: X~[193, n] = stacked per-expert masked
    copies of xrT (+ ones row).  out = X~.T @ G_t with G host-precomputed
    ([expert-placed W_emb rows] + pos_emb row), in float32r (1 cyc/row, ~1.5e-4).
    X~ rounded to f32r on ACT.  Two accumulating matmuls (K=128+65) per
    (tile, t); PSUM: 5 banks output rotation + router banks.
"""

import numpy as np
import ml_dtypes

# ---- problem constants (hardcoded; kernel.py must be self-contained) ----
B, C, S = 128, 32, 336
N_TOT = B * C            # 4096
R = S // 48              # 7
T = 6                    # patches per region after repeat
D = 512
N_CORES = 8
N_PER = N_TOT // N_CORES  # 512
NCHUNK = N_PER // 128     # 4
NJ = R * NCHUNK           # 28 tiles of 128 rows (row = r*512 + n_local)
PATCH_LENS = [8, 16, 24, 48]

_CACHE = {}


def _pos_emb(L, d):
    pos = np.arange(L, dtype=np.float64)[:, None]
    div = np.exp(np.arange(0, d, 2, dtype=np.float64) * -(np.log(10000.0) / d))
    pe = np.zeros((L, d), np.float32)
    pe[:, 0::2] = np.sin(pos * div).astype(np.float32)
    pe[:, 1::2] = np.cos(pos * div).astype(np.float32)
    return pe


def _repeat_q():
    q = np.zeros((4, T), np.int64)
    for e, plen in enumerate(PATCH_LENS):
        n = 48 // plen
        k = T - n + 1
        idx = np.arange(T) // k if k > 1 else np.arange(T)
        q[e] = np.minimum(idx, n - 1)
    return q


def _build():
    import concourse.bacc as bacc
    import concourse.mybir as mybir
    import concourse.tile as tile
    from concourse.masks import make_identity

    f32 = mybir.dt.float32
    f32r = mybir.dt.float32r
    bf16 = mybir.dt.bfloat16
    i32 = mybir.dt.int32
    u32 = mybir.dt.uint32

    nc = bacc.Bacc(trn_type="TRN2", target_bir_lowering=False)

    xa_d = nc.dram_tensor("xa", [128, NJ * 128], bf16, kind="ExternalInput")
    xal_d = nc.dram_tensor("xal", [48, NJ * 128], bf16, kind="ExternalInput")
    xb_d = nc.dram_tensor("xb", [64, NJ * 128], bf16, kind="ExternalInput")
    g_d = nc.dram_tensor("g", [128, NJ, 4], f32, kind="ExternalInput")
    w1h_d = nc.dram_tensor("w1h", [48, 64], bf16, kind="ExternalInput")
    w1l_d = nc.dram_tensor("w1l", [48, 64], bf16, kind="ExternalInput")
    b1_d = nc.dram_tensor("b1", [64, 1], f32, kind="ExternalInput")
    w2_d = nc.dram_tensor("w2", [64, 4], f32, kind="ExternalInput")
    ga_d = nc.dram_tensor("ga", [128, T, D], bf16, kind="ExternalInput")
    gb_d = nc.dram_tensor("gb", [66, R * T, D], bf16, kind="ExternalInput")
    ea_d = nc.dram_tensor("ea", [128, 1], f32, kind="ExternalInput")
    id_d = nc.dram_tensor("idm", [128, 128], bf16, kind="ExternalInput")
    eb_d = nc.dram_tensor("eb", [64, 1], f32, kind="ExternalInput")

    xp_d = nc.dram_tensor("xp", [N_PER, R * T, D], f32, kind="ExternalOutput")
    cls_d = nc.dram_tensor("cls", [R, N_PER], i32, kind="ExternalOutput")

    with tile.TileContext(nc) as tc:
        with (
            tc.tile_pool(name="sb", bufs=1) as sb,
            tc.tile_pool(name="sw", bufs=2) as sw,
            tc.tile_pool(name="ps", bufs=1, space="PSUM") as ps,
        ):
            # ---- resident loads (input ring = ACT HWDGE; outputs on SP ring) ----
            xa_s = sb.tile([128, NJ * 128], bf16)
            xal_s = sb.tile([48, NJ * 128], bf16)
            w1h_s = sb.tile([48, 64], bf16)
            w1l_s = sb.tile([48, 64], bf16)
            b1_s = sb.tile([64, 1], f32)
            w2_s = sb.tile([64, 4], f32)
            g_s = sb.tile([128, NJ, 4], f32)
            ea_s = sb.tile([128, 1], f32)
            eb_s = sb.tile([64, 1], f32)
            xb_s = sb.tile([64, NJ * 128], bf16)
            ga_s = sb.tile([128, T, D], bf16)
            ident = sb.tile([128, 128], bf16, name="ident")
            gb_all = sb.tile([66, R * T, D], bf16)
            qtr = NJ * 128 // 4
            nc.gpsimd.dma_start(xa_s[:, 0:qtr], xa_d[:, 0:qtr])
            nc.gpsimd.dma_start(w1h_s[:], w1h_d[:])
            nc.gpsimd.dma_start(w1l_s[:], w1l_d[:])
            nc.gpsimd.dma_start(b1_s[:], b1_d[:])
            nc.gpsimd.dma_start(w2_s[:], w2_d[:])
            nc.gpsimd.dma_start(ident[:], id_d[:])
            nc.gpsimd.dma_start(g_s[:], g_d[:])
            nc.gpsimd.dma_start(ea_s[:], ea_d[:])
            nc.gpsimd.dma_start(eb_s[:], eb_d[:])
            for _q in range(1, 4):
                nc.gpsimd.dma_start(
                    xa_s[:, _q * qtr:(_q + 1) * qtr], xa_d[:, _q * qtr:(_q + 1) * qtr]
                )
            nc.gpsimd.dma_start(xal_s[:], xal_d[:])
            nc.gpsimd.dma_start(ga_s[:], ga_d[:])
            nc.gpsimd.dma_start(xb_s[:], xb_d[:])
            nc.gpsimd.dma_start(gb_all[:], gb_d[:])

            idx_all = sb.tile([1, NJ * 128], bf16)
            idxf_all = sb.tile([128, NJ], bf16)
            cls_sb = sb.tile([128, NJ], i32)
            hT = sb.tile([64, NJ * 128], f32)

            # ---- router: fused m1 + per-j argmax chain per 512-chunk ----
            with nc.named_scope("router"):
              for jc in range(7):
                h_ps = ps.tile([64, 512], f32, tag="pout", bufs=4, name=f"hps{jc}")
                ch = slice(jc * 512, (jc + 1) * 512)
                nc.tensor.matmul(
                    h_ps[:], w1h_s[:], xa_s[0:48, ch], start=True, stop=False
                )
                nc.tensor.matmul(
                    h_ps[:], w1l_s[:], xa_s[0:48, ch], start=False, stop=False
                )
                nc.tensor.matmul(
                    h_ps[:], w1h_s[:], xal_s[:, ch], start=False, stop=True
                )
                nc.scalar.activation(
                    hT[:, jc * 512:(jc + 1) * 512], h_ps[:],
                    mybir.ActivationFunctionType.Relu, bias=b1_s[:, 0:1],
                )
                for j in range(jc * 4, jc * 4 + 4):
                    sl = slice(j * 128, (j + 1) * 128)
                    y0 = ps.tile([128, 4], f32, tag="pa", bufs=3, name=f"y0_{j}")
                    nc.tensor.matmul(
                        y0[:], hT[:, sl], w2_s[:], start=True, stop=True
                    )
                    y8 = sw.tile([128, 8], f32, tag="y8", bufs=4, name=f"y8_{j}")
                    nc.vector.memset(y8[:, 4:8], -1e30)
                    nc.vector.tensor_add(y8[:, 0:4], y0[:], g_s[:, j, :])
                    vmax = sw.tile([128, 8], f32, tag="vmax", bufs=4, name=f"vm{j}")
                    nc.vector.max(vmax[:], y8[:])
                    idx8 = sw.tile([128, 8], u32, tag="idx", bufs=4, name=f"ix{j}")
                    nc.vector.max_index(idx8[:], vmax[:], y8[:])
                    nc.vector.tensor_copy(
                        cls_sb[:, j:j + 1], idx8[:, 0:1].bitcast(i32)
                    )
                    nc.vector.tensor_copy(
                        idxf_all[:, j:j + 1], idx8[:, 0:1].bitcast(i32)
                    )

            with nc.named_scope("transposes"):
                for j in range(NJ):
                    sl = slice(j * 128, (j + 1) * 128)
                    ir_ps = ps.tile([1, 128], bf16, tag="pb", bufs=1, name=f"ir{j}")
                    nc.tensor.transpose(ir_ps[:], idxf_all[:, j:j + 1], ident[:])
                    nc.vector.tensor_copy(idx_all[:, sl], ir_ps[:])

            # ---- embed tiles ----
            with nc.named_scope("embed"):
                for j in range(NJ):
                    r, cn = j // NCHUNK, j % NCHUNK
                    sl = slice(j * 128, (j + 1) * 128)
                    idxb = sw.tile([128, 128], bf16, tag="idxb", bufs=4, name=f"ib{j}")
                    nc.gpsimd.partition_broadcast(idxb[:], idx_all[0:1, sl])
                    xta = sw.tile([128, 128], bf16, tag="xta", bufs=4, name=f"xa{j}")
                    nc.vector.scalar_tensor_tensor(
                        xta[:], idxb[:], ea_s[:, 0:1], xa_s[:, sl],
                        mybir.AluOpType.is_equal, mybir.AluOpType.mult,
                    )
                    xtb = sw.tile([66, 128], bf16, tag="xtb", bufs=4, name=f"xb{j}")
                    nc.vector.scalar_tensor_tensor(
                        xtb[0:64, :], idxb[0:64, :], eb_s[:, 0:1],
                        xb_s[:, sl],
                        mybir.AluOpType.is_equal, mybir.AluOpType.mult,
                    )
                    nc.vector.memset(xtb[64:66, :], 1.0)
                    out_sb = sw.tile([128, T, D], f32, tag="osb", bufs=6, name=f"os{j}")
                    for t in range(T):
                        o_t = ps.tile([128, D], f32, tag="pout", bufs=4,
                                      name=f"ops{j}_{t}")
                        nc.tensor.matmul(
                            o_t[:], xta[:], ga_s[:, t, :],
                            start=True, stop=False,
                        )
                        nc.tensor.matmul(
                            o_t[:], xtb[:], gb_all[:, r * T + t, :],
                            start=False, stop=True,
                        )
                        if t in (0, 3):
                            nc.vector.tensor_copy(out_sb[:, t, :], o_t[:])
                        else:
                            nc.scalar.copy(out_sb[:, t, :], o_t[:])
                    out_eng = nc.sync if (j % 2 == 0) else nc.scalar
                    out_eng.dma_start(
                        xp_d[cn * 128:(cn + 1) * 128, r * T:(r + 1) * T, :],
                        out_sb[:],
                    )
            nc.gpsimd.dma_start(
                cls_d.rearrange("r (c p) -> p r c", p=128),
                cls_sb[:].rearrange("p (r c) -> p r c", r=R),
            )

    nc.compile()
    return nc


def _host_prep(x, u_noise, W1, b1, W2, b2, W_embs):
    """Build per-core and shared input arrays."""
    q = _repeat_q()
    pos = _pos_emb(R * T, D)

    G = np.zeros((T, 193, D), np.float32)
    for e, plen in enumerate(PATCH_LENS):
        for t in range(T):
            r0 = e * 48 + q[e, t] * plen
            G[t, r0:r0 + plen, :] = W_embs[e]
    GA = np.ascontiguousarray(
        G[:, :128, :].transpose(1, 0, 2)
    ).astype(ml_dtypes.bfloat16)                      # [128, T, D]
    pos_hi = pos.astype(ml_dtypes.bfloat16)
    pos_lo = (pos - pos_hi.astype(np.float32)).astype(ml_dtypes.bfloat16)
    GB = np.zeros((66, R * T, D), ml_dtypes.bfloat16)  # [66, R*T, D]
    for r in range(R):
        for t in range(T):
            GB[:64, r * T + t] = G[t, 128:192].astype(ml_dtypes.bfloat16)
            GB[64, r * T + t] = pos_hi[r * T + t]
            GB[65, r * T + t] = pos_lo[r * T + t]

    eA = np.concatenate([np.full(48, 0), np.full(48, 1), np.full(32, 2)])
    eAc = np.ascontiguousarray(eA.astype(np.float32).reshape(128, 1))
    eB = np.concatenate([np.full(16, 2), np.full(48, 3)])
    eBc = np.ascontiguousarray(eB.astype(np.float32).reshape(64, 1))

    g_full = -np.log(-np.log(u_noise + np.float32(1e-10)) + np.float32(1e-10))
    g_full = (g_full + b2[None, None, :]).astype(np.float32)  # [N, R, 4]

    xs = np.ascontiguousarray(x.reshape(N_TOT, R, 48))
    kA = np.concatenate([np.arange(48), np.arange(48), np.arange(32)])
    kB = np.concatenate([np.arange(32, 48), np.arange(48)])

    in_maps = []
    for c in range(N_CORES):
        xc = xs[c * N_PER:(c + 1) * N_PER]            # [512, 7, 48]
        xa32 = np.ascontiguousarray(
            xc[:, :, kA].transpose(2, 1, 0).reshape(128, NJ * 128)
        )
        xa = xa32.astype(ml_dtypes.bfloat16)
        xal = (xa32[0:48] - xa[0:48].astype(np.float32)).astype(ml_dtypes.bfloat16)
        xb = np.ascontiguousarray(
            xc[:, :, kB].transpose(2, 1, 0).reshape(64, NJ * 128)
        ).astype(ml_dtypes.bfloat16)
        gc = g_full[c * N_PER:(c + 1) * N_PER]        # [512, 7, 4]
        gcr = np.ascontiguousarray(
            gc.transpose(1, 0, 2).reshape(NJ, 128, 4).transpose(1, 0, 2)
        )                                             # [128, NJ, 4]
        W1h = W1.astype(ml_dtypes.bfloat16)
        W1l = (W1 - W1h.astype(np.float32)).astype(ml_dtypes.bfloat16)
        in_maps.append({
            "xa": xa, "xal": xal, "xb": xb, "g": gcr,
            "idm": np.eye(128, dtype=ml_dtypes.bfloat16),
            "w1h": np.ascontiguousarray(W1h),
            "w1l": np.ascontiguousarray(W1l),
            "b1": np.ascontiguousarray(b1.reshape(64, 1)),
            "w2": np.ascontiguousarray(W2),
            "ga": GA, "gb": GB, "ea": eAc, "eb": eBc,
        })
    return in_maps


def kernel(x, u_noise, W1, b1, W2, b2, W_emb0, W_emb1, W_emb2, W_emb3,
           _want_trace=False):
    from concourse.bass_utils import run_bass_kernel_spmd

    x = np.asarray(x, np.float32)
    u_noise = np.asarray(u_noise, np.float32)
    W1 = np.asarray(W1, np.float32)
    b1 = np.asarray(b1, np.float32)
    W2 = np.asarray(W2, np.float32)
    b2 = np.asarray(b2, np.float32)
    W_embs = [np.asarray(w, np.float32) for w in (W_emb0, W_emb1, W_emb2, W_emb3)]

    if "nc" not in _CACHE:
        _CACHE["nc"] = _build()
    nc = _CACHE["nc"]

    in_maps = _host_prep(x, u_noise, W1, b1, W2, b2, W_embs)
    res = run_bass_kernel_spmd(
        nc, in_maps, core_ids=list(range(N_CORES)), trace=_want_trace
    )
    if _want_trace:
        _CACHE["last_result"] = res

    x_patch = np.concatenate(
        [res.results[c]["xp"] for c in range(N_CORES)], axis=0
    )
    cls_full = np.concatenate(
        [res.results[c]["cls"] for c in range(N_CORES)], axis=1
    )
    cls_pred = np.ascontiguousarray(cls_full.reshape(-1))
    return x_patch, np.int32(C), cls_pred
